# revision 1
# baseline (speedup 1.0000x reference)
"""Trainium2 Bass kernel for nn_M04AdaptiveVQ.

Data-parallel over B: each of the 8 NeuronCores processes one batch element
(1500 frames). Centroid bank, MLP weights and RVQ codebooks are replicated.

Per-core pipeline (all layouts feature-major (feature, frame) unless noted):
  1. VQ: ndist[t,k] = 2*x_t.c_k - (|c_k|^2 - 1024) via f32r matmuls
     (16 k-chunks of 512, streamed), argmax via DVE max8/max_index,
     two-level combine -> per-frame centroid index.
  2. Gather centroid rows (SWDGE dma_gather) -> PE-transpose to (C,T).
  3. spk_raw = feature - lin_dec; enc MLP (Lrelu on ACT).
  4. nrm MLP (Relu); spk_enc_norm = spk_enc / (norm+eps).
  5. RVQ: 4 stages of 1024-bin nearest-codebook search + gather + residual
     update; q_sum accumulated exactly from gathered rows.
  6. dec MLP; out = lin_dec + spk_dec.
"""
import sys
if '/opt/trn_rl_repo' not in sys.path:
    sys.path.insert(0, '/opt/trn_rl_repo')
import numpy as np

B, C, T = 8, 1024, 1500
K = 8192
D, H = 256, 512
NQ, BINS = 4, 1024
TPAD = 1536
NFT = TPAD // 128      # 12 frame tiles
NKC = K // 512         # 16 centroid chunks
NCC = C // 128         # 8
NDC = D // 128         # 2
NHC = H // 128         # 4
NTT = TPAD // 512      # 3 mlp t-tiles
EPS = 1e-8
CSHIFT = float(C)      # centers |c|^2 for f32r precision
CBSHIFT = float(D)

TRACE = False
DEBUG = False
LAST_RESULT = None
_prog = None


def _build_program():
    import concourse.bacc as bacc
    import concourse.mybir as mybir
    from concourse.tile import TileContext
    from concourse.masks import make_identity
    from contextlib import ExitStack

    f32 = mybir.dt.float32
    f32r = mybir.dt.float32r
    i16 = mybir.dt.int16
    u16 = mybir.dt.uint16
    AF = mybir.ActivationFunctionType
    OP = mybir.AluOpType
    AX = mybir.AxisListType

    nc = bacc.Bacc("TRN2", target_bir_lowering=False, debug=False, num_devices=8)

    dfeat = nc.dram_tensor("feat", [C, TPAD], f32, kind="ExternalInput")
    dcentT2 = nc.dram_tensor("centT2", [C, K], f32, kind="ExternalInput")
    dcnc = nc.dram_tensor("cnc", [128, K], f32, kind="ExternalInput")
    dcent = nc.dram_tensor("cent", [K, C], f32, kind="ExternalInput")
    dcent_ext = nc.dram_tensor("cent_ext", [K, 1088], f32, kind="ExternalInput")
    dbase = nc.dram_tensor("basegidx", [128, 128], f32, kind="ExternalInput")
    dw = {}
    for pre, shapes in (
        ("enc", [(C, H), (H, H), (H, D)]),
        ("nrm", [(C, H), (H, H), (H, D)]),
        ("dec", [(D, H), (H, H), (H, C)]),
    ):
        for i, (ni, no) in enumerate(shapes):
            dw[f"{pre}_w{i}"] = nc.dram_tensor(f"{pre}_w{i}", [ni, no], f32,
                                               kind="ExternalInput")
            dw[f"{pre}_b{i}"] = nc.dram_tensor(f"{pre}_b{i}", [no], f32,
                                               kind="ExternalInput")
    dcbT2 = nc.dram_tensor("cbT2", [NQ, D, BINS], f32, kind="ExternalInput")
    dcbnc = nc.dram_tensor("cbnc", [128, NQ, BINS], f32, kind="ExternalInput")
    dcb = nc.dram_tensor("cb", [NQ, BINS, D], f32, kind="ExternalInput")
    dout = nc.dram_tensor("out", [C, T], f32, kind="ExternalOutput")
    if DEBUG:
        ddbg_gi1 = nc.dram_tensor("dbg_gi1", [128, NFT], mybir.dt.uint16,
                                  kind="ExternalOutput")
        ddbg_gi2 = nc.dram_tensor("dbg_gi2", [128, NFT], mybir.dt.uint16,
                                  kind="ExternalOutput")
        ddbg_gsel = nc.dram_tensor("dbg_gsel", [128, NFT], mybir.dt.uint16,
                                   kind="ExternalOutput")
        ddbg_delta = nc.dram_tensor("dbg_delta", [128, NFT], f32,
                                    kind="ExternalOutput")
        ddbg_sdot = nc.dram_tensor("dbg_sdot", [128, NFT], f32,
                                   kind="ExternalOutput")

    def wrap_idx(gidxs, idxw):
        # gidxs (128, NFT) u16: [p, j] = idx of frame j*128+p
        # idxw (128, 96) i16: [q, 8a+b] = idx of frame a*128 + 16b+q
        idxw3 = idxw[:].rearrange("p (a b) -> p a b", b=8)
        for b in range(8):
            nc.sync.dma_start(out=idxw3[0:16, :, b],
                              in_=gidxs[16 * b:16 * b + 16, :].bitcast(i16))
        for g in range(1, 8):
            nc.sync.dma_start(out=idxw[16 * g:16 * g + 16, :], in_=idxw[0:16, :])

    def load_w(pool, pre, i, ni, no):
        a, b = ni // 128, no // 128
        wt = pool.tile([128, a, b, 128], f32r, tag=f"{pre}_w{i}")
        nc.sync.dma_start(
            out=wt[:],
            in_=dw[f"{pre}_w{i}"][:].rearrange("(a p) (b q) -> p a b q",
                                               p=128, q=128).bitcast(f32r))
        bt = pool.tile([128, b], f32, tag=f"{pre}_b{i}")
        nc.sync.dma_start(out=bt[:],
                          in_=dw[f"{pre}_b{i}"][:].rearrange("(a p) -> p a", p=128))
        return wt, bt

    def mlp_layer(mp, wt, bt, in_t, out_t, ic, oc, func, alpha=0.0):
        # in_t (128, ic, 512) f32r; out_t slices (128, oc, 512)
        for hc in range(oc):
            ps = mp.tile([128, 512], f32, tag="mlp_ps")
            for cc in range(ic):
                nc.tensor.matmul(ps[:], wt[:, cc, hc], in_t[:, cc],
                                 start=(cc == 0), stop=(cc == ic - 1))
            nc.scalar.activation(out_t[:, hc], ps[:], func,
                                 bias=bt[:, hc:hc + 1], scale=1.0, alpha=alpha)

    with TileContext(nc) as tc:
        with ExitStack() as top:
            const = top.enter_context(tc.tile_pool(name="const", bufs=1))
            ident = const.tile([128, 128], f32)
            make_identity(nc, ident[:])
            ones_f = const.tile([1, 128], f32)
            nc.vector.memset(ones_f[:], 1.0)
            ones_r = const.tile([1, 128], f32r)
            nc.vector.tensor_copy(ones_r[:], ones_f[:])
            base_t = const.tile([128, 128], f32)
            nc.sync.dma_start(out=base_t[:], in_=dbase[:])
            gidxs = const.tile([128, NFT], u16)
            idxw = const.tile([128, 96], i16)
            gi1 = const.tile([128, NFT], u16)
            gi2 = const.tile([128, NFT], u16)

            # ---------------- Phase 1: VQ distance + argmax ----------------
            with tc.tile_pool(name="featp", bufs=1) as featp:
                feat_t = featp.tile([128, NCC, TPAD], f32r)
                nc.sync.dma_start(
                    out=feat_t[:],
                    in_=dfeat[:].rearrange("(a p) t -> p a t", p=128).bitcast(f32r))
                with tc.tile_pool(name="cw", bufs=3) as cw, \
                     tc.tile_pool(name="vqps", bufs=6, space="PSUM") as vqps, \
                     tc.tile_pool(name="vqst", bufs=4) as vqst, \
                     tc.tile_pool(name="win", bufs=1) as win, \
                     tc.tile_pool(name="comb", bufs=1) as comb:
                    val8 = win.tile([128, NFT, 128], f32)
                    idx8 = win.tile([128, NFT, 128], u16)
                    for kc in range(NKC):
                        ks = slice(kc * 512, (kc + 1) * 512)
                        cwt = cw.tile([128, NCC, 512], f32r, tag="cw")
                        nc.sync.dma_start(
                            out=cwt[:],
                            in_=dcentT2[:, ks].rearrange("(a p) k -> p a k",
                                                         p=128).bitcast(f32r))
                        cncc = cw.tile([128, 512], f32, tag="cncc")
                        nc.sync.dma_start(out=cncc[:], in_=dcnc[:, ks])
                        for ft in range(NFT):
                            fs = slice(ft * 128, (ft + 1) * 128)
                            ps = vqps.tile([128, 512], f32, tag="vqps")
                            for cc in range(NCC):
                                nc.tensor.matmul(ps[:], feat_t[:, cc, fs],
                                                 cwt[:, cc],
                                                 start=(cc == 0),
                                                 stop=(cc == NCC - 1))
                            dist = vqst.tile([128, 512], f32, tag="dist")
                            nc.vector.tensor_tensor(out=dist[:], in0=ps[:],
                                                    in1=cncc[:], op=OP.add)
                            v8s = val8[:, ft, kc * 8:(kc + 1) * 8]
                            nc.vector.max(out=v8s, in_=dist[:])
                            nc.vector.max_index(
                                out=idx8[:, ft, kc * 8:(kc + 1) * 8],
                                in_max=v8s, in_values=dist[:])
                    # two-level combine -> rank-0/1 global index per frame
                    for ft in range(NFT):
                        idxf = comb.tile([128, 128], f32, tag="idxf")
                        nc.vector.tensor_copy(idxf[:], idx8[:, ft])
                        nc.vector.tensor_tensor(out=idxf[:], in0=idxf[:],
                                                in1=base_t[:], op=OP.add)
                        g8 = comb.tile([128, 8], f32, tag="g8")
                        nc.vector.max(out=g8[:], in_=val8[:, ft])
                        for rank, gi in ((0, gi1), (1, gi2)):
                            msk = comb.tile([128, 128], f32, tag="msk")
                            nc.vector.tensor_scalar(msk[:], val8[:, ft],
                                                    g8[:, rank:rank + 1],
                                                    None, op0=OP.is_equal)
                            nc.vector.tensor_tensor(out=msk[:], in0=msk[:],
                                                    in1=idxf[:], op=OP.mult)
                            gx = comb.tile([128, 1], f32, tag="gx")
                            nc.vector.reduce_max(gx[:], msk[:], axis=AX.X)
                            nc.vector.tensor_copy(gi[:, ft:ft + 1], gx[:])

                # exact fp32 rescore of the two candidates per frame
                with tc.tile_pool(name="rsc", bufs=2) as rsc, \
                     tc.tile_pool(name="rsi", bufs=1) as rsi, \
                     tc.tile_pool(name="rscps", bufs=2, space="PSUM") as rscps:
                    idxw1 = rsi.tile([128, 96], i16)
                    idxw2 = rsi.tile([128, 96], i16)
                    wrap_idx(gi1, idxw1)
                    wrap_idx(gi2, idxw2)
                    for ft in range(NFT):
                        fs = slice(ft * 128, (ft + 1) * 128)
                        g1t = rsc.tile([128, 1, 1088], f32, tag="g1t")
                        nc.gpsimd.dma_gather(out_ap=g1t[:], in_ap=dcent_ext[:],
                                             idxs_ap=idxw1[:, 8 * ft:8 * ft + 8],
                                             num_idxs=128, num_idxs_reg=128,
                                             elem_size=1088)
                        g2t = rsc.tile([128, 1, 1088], f32, tag="g2t")
                        nc.gpsimd.dma_gather(out_ap=g2t[:], in_ap=dcent_ext[:],
                                             idxs_ap=idxw2[:, 8 * ft:8 * ft + 8],
                                             num_idxs=128, num_idxs_reg=128,
                                             elem_size=1088)
                        cd = rsc.tile([128, 1024], f32, tag="cd")
                        nc.vector.tensor_tensor(out=cd[:], in0=g1t[:, 0, 0:1024],
                                                in1=g2t[:, 0, 0:1024],
                                                op=OP.subtract)
                        # cdiff to c-major, then exact fp32 PE dot:
                        # diag(cdT.T @ x) = per-frame x . (c1 - c2)
                        cdT = rsc.tile([128, NCC, 128], f32, tag="cdT")
                        for cc in range(NCC):
                            pt = rscps.tile([128, 128], f32, tag="rscps")
                            nc.tensor.transpose(
                                pt[:], cd[:, cc * 128:(cc + 1) * 128], ident[:])
                            nc.vector.tensor_copy(cdT[:, cc], pt[:])
                        xf = rsc.tile([128, NCC, 128], f32, tag="xf")
                        nc.sync.dma_start(
                            out=xf[:],
                            in_=dfeat[:, fs].rearrange("(a p) t -> p a t", p=128))
                        ps2 = rscps.tile([128, 128], f32, tag="rscmm")
                        for cc in range(NCC):
                            nc.tensor.matmul(ps2[:], cdT[:, cc], xf[:, cc],
                                             start=(cc == 0), stop=(cc == NCC - 1))
                        dg = rsc.tile([128, 128], f32, tag="dg")
                        nc.vector.tensor_tensor(out=dg[:], in0=ps2[:],
                                                in1=ident[:], op=OP.mult)
                        sdot = rsc.tile([128, 1], f32, tag="sdot")
                        nc.vector.reduce_sum(sdot[:], dg[:], axis=AX.X)
                        delta = rsc.tile([128, 1], f32, tag="delta")
                        nc.vector.tensor_scalar_mul(delta[:], sdot[:], -2.0)
                        nd = rsc.tile([128, 1], f32, tag="nd")
                        nc.vector.tensor_tensor(out=nd[:],
                                                in0=g1t[:, 0, 1024:1025],
                                                in1=g2t[:, 0, 1024:1025],
                                                op=OP.subtract)
                        nc.vector.tensor_tensor(out=delta[:], in0=delta[:],
                                                in1=nd[:], op=OP.add)
                        selm = rsc.tile([128, 1], mybir.dt.uint32, tag="selm")
                        nc.vector.tensor_scalar(selm[:], delta[:], 0.0, None,
                                                op0=OP.is_gt)
                        nc.vector.tensor_copy(gidxs[:, ft:ft + 1],
                                              gi1[:, ft:ft + 1])
                        nc.vector.copy_predicated(gidxs[:, ft:ft + 1], selm[:],
                                                  gi2[:, ft:ft + 1])
                        if DEBUG:
                            nc.sync.dma_start(out=ddbg_delta[:, ft:ft + 1],
                                              in_=delta[:])
                            nc.sync.dma_start(out=ddbg_sdot[:, ft:ft + 1],
                                              in_=sdot[:])

            if DEBUG:
                nc.sync.dma_start(out=ddbg_gi1[:], in_=gi1[:])
                nc.sync.dma_start(out=ddbg_gi2[:], in_=gi2[:])
                nc.sync.dma_start(out=ddbg_gsel[:], in_=gidxs[:])
            wrap_idx(gidxs, idxw)

            ld_s = ExitStack()
            ldp = ld_s.enter_context(tc.tile_pool(name="ldp", bufs=1))
            ld_t = ldp.tile([128, NCC, TPAD], f32)        # lin_dec, (C, T)

            # ---------------- Phase 2: gather + transpose lin_dec ----------
            with tc.tile_pool(name="lg", bufs=2) as lg, \
                 tc.tile_pool(name="trps", bufs=2, space="PSUM") as trps:
                for ft in range(NFT):
                    g = lg.tile([128, 1, 1024], f32, tag="lg")
                    nc.gpsimd.dma_gather(out_ap=g[:], in_ap=dcent[:],
                                         idxs_ap=idxw[:, 8 * ft:8 * ft + 8],
                                         num_idxs=128, num_idxs_reg=128,
                                         elem_size=1024)
                    for cc in range(NCC):
                        pt = trps.tile([128, 128], f32, tag="trps")
                        nc.tensor.transpose(pt[:],
                                            g[:, 0, cc * 128:(cc + 1) * 128],
                                            ident[:])
                        nc.vector.tensor_copy(
                            ld_t[:, cc, ft * 128:(ft + 1) * 128], pt[:])

            sed_s = ExitStack()
            sedp = sed_s.enter_context(tc.tile_pool(name="sedp", bufs=1))
            sed = sedp.tile([128, NDC, TPAD], f32r)   # q_sum * norm_vec

            mid_s = ExitStack()
            midp = mid_s.enter_context(tc.tile_pool(name="midp", bufs=1))
            spk_enc = midp.tile([128, NDC, TPAD], f32)
            norm_v = midp.tile([128, NDC, TPAD], f32)

            # ---------------- Phase 3: spk_raw + enc MLP ----------------
            with tc.tile_pool(name="wenc", bufs=1) as wenc:
                we0, be0 = load_w(wenc, "enc", 0, C, H)
                we1, be1 = load_w(wenc, "enc", 1, H, H)
                we2, be2 = load_w(wenc, "enc", 2, H, D)
                with tc.tile_pool(name="henc", bufs=1) as hp, \
                     tc.tile_pool(name="ftmp", bufs=2) as ftmp, \
                     tc.tile_pool(name="mlpps", bufs=6, space="PSUM") as mp:
                    for tt in range(NTT):
                        ts_ = slice(tt * 512, (tt + 1) * 512)
                        spk_tt = hp.tile([128, NCC, 512], f32r, tag="spk_tt")
                        for cc in range(NCC):
                            fre = ftmp.tile([128, 512], f32, tag="fre")
                            nc.sync.dma_start(
                                out=fre[:],
                                in_=dfeat[cc * 128:(cc + 1) * 128, ts_])
                            nc.vector.tensor_tensor(out=spk_tt[:, cc],
                                                    in0=fre[:],
                                                    in1=ld_t[:, cc, ts_],
                                                    op=OP.subtract)
                        h0 = hp.tile([128, NHC, 512], f32r, tag="h0")
                        mlp_layer(mp, we0, be0, spk_tt, h0,
                                  NCC, NHC, AF.Lrelu, alpha=0.01)
                        h1 = hp.tile([128, NHC, 512], f32r, tag="h1")
                        mlp_layer(mp, we1, be1, h0, h1, NHC, NHC,
                                  AF.Lrelu, alpha=0.01)
                        mlp_layer(mp, we2, be2, h1, spk_enc[:, :, ts_],
                                  NHC, NDC, AF.Identity)

            # ---------------- Phase 3.5: nrm MLP ----------------
            with tc.tile_pool(name="wnrm", bufs=1) as wnrm:
                wn0, bn0 = load_w(wnrm, "nrm", 0, C, H)
                wn1, bn1 = load_w(wnrm, "nrm", 1, H, H)
                wn2, bn2 = load_w(wnrm, "nrm", 2, H, D)
                with tc.tile_pool(name="hnrm", bufs=1) as hp, \
                     tc.tile_pool(name="ldrp", bufs=1) as ldrp, \
                     tc.tile_pool(name="mlpps2", bufs=6, space="PSUM") as mp:
                    for tt in range(NTT):
                        ts_ = slice(tt * 512, (tt + 1) * 512)
                        ldr = ldrp.tile([128, NCC, 512], f32r, tag="ldr")
                        nc.vector.tensor_copy(ldr[:], ld_t[:, :, ts_])
                        n0 = hp.tile([128, NHC, 512], f32r, tag="n0")
                        mlp_layer(mp, wn0, bn0, ldr, n0, NCC, NHC, AF.Relu)
                        n1 = hp.tile([128, NHC, 512], f32r, tag="n1")
                        mlp_layer(mp, wn1, bn1, n0, n1, NHC, NHC, AF.Relu)
                        mlp_layer(mp, wn2, bn2, n1, norm_v[:, :, ts_],
                                  NHC, NDC, AF.Relu)

            # ---------------- Phase 4: normalize + RVQ ----------------
            rq_s = ExitStack()
            rqp = rq_s.enter_context(tc.tile_pool(name="rqp", bufs=1))
            r_t = rqp.tile([128, NDC, TPAD], f32r)
            qs = rqp.tile([128, NDC, TPAD], f32)
            with tc.tile_pool(name="tmpp", bufs=1) as tmpp:
                recip = tmpp.tile([128, NDC, TPAD], f32)
                nc.vector.tensor_scalar_add(recip[:], norm_v[:], EPS)
                nc.vector.reciprocal(recip[:], recip[:])
                sen = tmpp.tile([128, NDC, TPAD], f32)
                nc.vector.tensor_tensor(out=sen[:], in0=spk_enc[:], in1=recip[:],
                                        op=OP.mult)
                nc.vector.tensor_copy(r_t[:], sen[:])
                nc.vector.memset(qs[:], 0.0)

            with tc.tile_pool(name="cbidx", bufs=1) as cbidx, \
                 tc.tile_pool(name="cbp", bufs=2) as cbp, \
                 tc.tile_pool(name="rps", bufs=2, space="PSUM") as rps, \
                 tc.tile_pool(name="rst", bufs=2) as rst, \
                 tc.tile_pool(name="rcomb", bufs=1) as rcomb, \
                 tc.tile_pool(name="qg", bufs=2) as qg, \
                 tc.tile_pool(name="trps2", bufs=2, space="PSUM") as trps2:
                gidxr = cbidx.tile([128, NFT], u16)
                idxwr = cbidx.tile([128, 96], i16)
                for q in range(NQ):
                    cbt = cbp.tile([128, NDC, BINS], f32r, tag="cbt")
                    nc.sync.dma_start(
                        out=cbt[:],
                        in_=dcbT2[q].rearrange("(a p) n -> p a n",
                                               p=128).bitcast(f32r))
                    cbn = cbp.tile([128, BINS], f32, tag="cbn")
                    nc.sync.dma_start(out=cbn[:], in_=dcbnc[:, q, :])
                    for ft in range(NFT):
                        fs = slice(ft * 128, (ft + 1) * 128)
                        ps = rps.tile([128, BINS], f32, tag="rps")
                        for half in range(2):
                            hs = slice(half * 512, (half + 1) * 512)
                            for dc in range(NDC):
                                nc.tensor.matmul(ps[:, hs], r_t[:, dc, fs],
                                                 cbt[:, dc, hs],
                                                 start=(dc == 0),
                                                 stop=(dc == NDC - 1))
                        dist = rst.tile([128, BINS], f32, tag="rdist")
                        nc.vector.tensor_tensor(out=dist[:], in0=ps[:],
                                                in1=cbn[:], op=OP.add)
                        v8 = rcomb.tile([128, 8], f32, tag="rv8")
                        nc.vector.max(out=v8[:], in_=dist[:])
                        i8 = rcomb.tile([128, 8], u16, tag="ri8")
                        nc.vector.max_index(out=i8[:], in_max=v8[:],
                                            in_values=dist[:])
                        nc.vector.tensor_copy(gidxr[:, ft:ft + 1], i8[:, 0:1])
                    wrap_idx(gidxr, idxwr)
                    for ft in range(NFT):
                        fs = slice(ft * 128, (ft + 1) * 128)
                        gq = qg.tile([128, 1, 256], f32, tag="gq")
                        nc.gpsimd.dma_gather(out_ap=gq[:], in_ap=dcb[q],
                                             idxs_ap=idxwr[:, 8 * ft:8 * ft + 8],
                                             num_idxs=128, num_idxs_reg=128,
                                             elem_size=256)
                        for dc in range(NDC):
                            pt = trps2.tile([128, 128], f32, tag="trps2")
                            nc.tensor.transpose(
                                pt[:], gq[:, 0, dc * 128:(dc + 1) * 128],
                                ident[:])
                            nc.vector.tensor_tensor(out=qs[:, dc, fs],
                                                    in0=qs[:, dc, fs],
                                                    in1=pt[:], op=OP.add)
                            if q < NQ - 1:
                                nc.vector.tensor_tensor(
                                    out=r_t[:, dc, fs],
                                    in0=r_t[:, dc, fs].bitcast(f32),
                                    in1=pt[:], op=OP.subtract)

            # spk_enc_denorm = q_sum * norm_vec (per t-tile: lets dec start
            # before the last RVQ frame-tiles finish)
            for tt in range(NTT):
                ts_ = slice(tt * 512, (tt + 1) * 512)
                nc.vector.tensor_tensor(out=sed[:, :, ts_], in0=qs[:, :, ts_],
                                        in1=norm_v[:, :, ts_], op=OP.mult)
            rq_s.close()
            mid_s.close()

            # ---------------- Phase 5: dec MLP + final add ----------------
            with tc.tile_pool(name="wdec", bufs=1) as wdec:
                wd0, bd0 = load_w(wdec, "dec", 0, D, H)
                wd1, bd1 = load_w(wdec, "dec", 1, H, H)
                wd2, bd2 = load_w(wdec, "dec", 2, H, C)
                with tc.tile_pool(name="hdec", bufs=1) as hp, \
                     tc.tile_pool(name="outp", bufs=3) as outp, \
                     tc.tile_pool(name="mlpps3", bufs=6, space="PSUM") as mp:
                    for tt in range(NTT):
                        ts_ = slice(tt * 512, (tt + 1) * 512)
                        d0 = hp.tile([128, NHC, 512], f32r, tag="d0")
                        mlp_layer(mp, wd0, bd0, sed[:, :, ts_], d0, NDC, NHC,
                                  AF.Lrelu, alpha=0.01)
                        d1 = hp.tile([128, NHC, 512], f32r, tag="d1")
                        mlp_layer(mp, wd1, bd1, d0, d1, NHC, NHC,
                                  AF.Lrelu, alpha=0.01)
                        n = min(512, T - tt * 512)
                        for hc in range(NCC):
                            ps = mp.tile([128, 512], f32, tag="mlp_ps")
                            for cc in range(NHC):
                                nc.tensor.matmul(ps[:], wd2[:, cc, hc],
                                                 d1[:, cc], start=(cc == 0),
                                                 stop=(cc == NHC - 1))
                            tmpo = outp.tile([128, 512], f32, tag="tmpo")
                            nc.scalar.activation(tmpo[:], ps[:], AF.Identity,
                                                 bias=bd2[:, hc:hc + 1],
                                                 scale=1.0)
                            nc.vector.tensor_tensor(out=tmpo[:], in0=tmpo[:],
                                                    in1=ld_t[:, hc, ts_],
                                                    op=OP.add)
                            nc.sync.dma_start(
                                out=dout[:].rearrange("(a p) t -> p a t", p=128)
                                    [:, hc, tt * 512:tt * 512 + n],
                                in_=tmpo[:, 0:n])
            sed_s.close()
            ld_s.close()

    nc.compile()
    return nc


def _get_program():
    global _prog
    if _prog is None:
        _prog = _build_program()
    return _prog


def _cent_ext(centroid):
    ce = np.zeros((K, 1088), dtype=np.float32)
    ce[:, :C] = centroid
    ce[:, C] = (centroid.astype(np.float64) ** 2).sum(1).astype(np.float32)
    return ce


def _host_prep(inputs):
    g = lambda k: np.ascontiguousarray(np.asarray(inputs[k], dtype=np.float32))
    feature = g('feature')               # (B, C, T)
    centroid = g('centroid')             # (K, C)
    codebooks = g('codebooks')           # (NQ, BINS, D)

    feats = np.zeros((B, C, TPAD), dtype=np.float32)
    feats[:, :, :T] = feature

    shared = {
        "centT2": np.ascontiguousarray(2.0 * centroid.T),
        "cnc": np.ascontiguousarray(np.broadcast_to(
            (CSHIFT - (centroid.astype(np.float64) ** 2).sum(1)
             ).astype(np.float32)[None, :], (128, K))),
        "cent": centroid,
        "cent_ext": _cent_ext(centroid),
        "basegidx": np.broadcast_to(
            (512.0 * (np.arange(128) // 8)).astype(np.float32), (128, 128)
        ).copy(),
        "cbT2": np.ascontiguousarray(2.0 * codebooks.transpose(0, 2, 1)),
        "cbnc": np.ascontiguousarray(np.broadcast_to(
            (CBSHIFT - (codebooks.astype(np.float64) ** 2).sum(-1)
             ).astype(np.float32).reshape(1, NQ, BINS), (128, NQ, BINS))),
        "cb": codebooks,
    }
    for pre in ("enc", "nrm", "dec"):
        for i in range(3):
            shared[f"{pre}_w{i}"] = g(f"{pre}_w{i}")
            shared[f"{pre}_b{i}"] = g(f"{pre}_b{i}")

    in_maps = []
    for b in range(B):
        m = dict(shared)
        m["feat"] = np.ascontiguousarray(feats[b])
        in_maps.append(m)
    return in_maps


def kernel(**inputs):
    global LAST_RESULT
    from concourse.bass_utils import run_bass_kernel_spmd
    nc = _get_program()
    in_maps = _host_prep(inputs)
    kwargs = {}
    if TRACE:
        try:
            from ntff_shim import install_ntff_hook
            install_ntff_hook()
            kwargs["trace"] = True
        except Exception:
            pass
    res = run_bass_kernel_spmd(nc, in_maps, core_ids=list(range(B)), **kwargs)
    LAST_RESULT = res
    out = np.empty((B, C, T), dtype=np.float32)
    for b in range(B):
        out[b] = res.results[b]["out"]
    return out

